# revision 15
# baseline (speedup 1.0000x reference)
"""Multi-head attention (per-head full-embed projections) on 8 TRN2 NeuronCores.

Problem (hardcoded shapes):
    x      [8, 1024, 768] f32
    qkv_w  [12, 2304, 768] f32   (per-head Linear(E, 3E) torch weight)
    qkv_b  [12, 2304] f32
    out_w  [768, 9216] f32
    out_b  [768] f32
    out    [8, 1024, 768] f32

Sharding: data-parallel over batch (B=8 -> 1 batch element per core).
No collectives. Host pre-transposes/casts weights+activations (free; not in
HW time).

v2 design notes (vs the f32r spill-to-DRAM baseline at ~1.5ms):
  * All matmul operands in bf16 (PSUM accumulation stays f32). Same PE rate
    (1 cycle/row) but halves LDWEIGHTS time + SBUF footprint + HBM traffic.
    Simulated end-to-end L2 error ~3e-3 (gate is 2e-2).
  * Final projection fused into phase A per head: out accumulates in an
    SBUF f32 accumulator via DVE adds; the 75MB oT spill and the DMA-bound
    phase B tail are gone.
  * K bias dropped: softmax over keys is invariant to the (Q_q+bQ).bK
    per-row constant, so only Q needs its bias. V bias folds into the final
    bias on host (rows of P/r sum to 1).
  * Softmax denominator path (DVE tree-sum -> GPSIMD partition_all_reduce
    -> DVE reciprocal, ~12us latency) is hidden by deferring the fused
    projection of each q-half behind the next score/AV block in PE order.

Per-core PE stream / head: [fusedB(h-1,q1)] [QK proj] [V proj] [scores q0]
[AV q0] [scores q1] [fusedB q0] [AV q1] -> next head. ~2.96M PE cycles
total = 1.23ms floor at 2.4GHz.
"""

import numpy as np

B, S, E, H = 8, 1024, 768, 12
F3 = 3 * E                 # 2304
TE = E // 128              # 6  e-tiles
TS = S // 128              # 8  s-tiles
HE = H * E                 # 9216
SCALE = 1.0 / float(np.sqrt(E))

_BUILT = None


def _build():
    import concourse.bacc as bacc
    import concourse.tile as tile
    import concourse.mybir as mybir
    import concourse.bass_isa as bass_isa

    F32 = mybir.dt.float32
    F32R = mybir.dt.float32r
    BF16 = mybir.dt.bfloat16
    Exp = mybir.ActivationFunctionType.Exp

    nc = bacc.Bacc("TRN2", target_bir_lowering=False, debug=False)

    xT_d = nc.dram_tensor("xT", [E, S], BF16, kind="ExternalInput")
    w_d = nc.dram_tensor("wqkvT", [H, E, F3], BF16, kind="ExternalInput")
    owT_d = nc.dram_tensor("owT", [HE, E], BF16, kind="ExternalInput")
    qb_d = nc.dram_tensor("qb", [128, H * TE], F32, kind="ExternalInput")
    fb_d = nc.dram_tensor("fb", [1, E], BF16, kind="ExternalInput")
    onesr_d = nc.dram_tensor("onesr", [1, 128], BF16, kind="ExternalInput")
    out_d = nc.dram_tensor("out", [S, E], F32, kind="ExternalOutput")

    with tile.TileContext(nc) as tc:
        with (
            nc.allow_low_precision(reason="bf16 matmul pipeline"),
            tc.tile_pool(name="persist", bufs=1) as persist,
        ):
            # ---- persistent tiles ----
            xt = persist.tile([128, TE, S], BF16, tag="xt")
            xTr = xT_d.rearrange("(t p) s -> p t s", p=128)
            for et in range(TE):
                nc.sync.dma_start(xt[:, et, :], xTr[:, et, :])
            qb = persist.tile([128, H * TE], F32, tag="qb")
            nc.sync.dma_start(qb[:], qb_d[:])
            fb = persist.tile([1, E], BF16, tag="fb")
            nc.sync.dma_start(fb[:], fb_d[:])
            onesr = persist.tile([1, 128], BF16, tag="onesr")
            nc.sync.dma_start(onesr[:], onesr_d[:])
            # out accumulator [s-tile, g]; initialized by head 0 (copy)
            acc = persist.tile([128, TS, E], F32, tag="acc")

            with (
                tc.tile_pool(name="wp", bufs=12) as wp,
                tc.tile_pool(name="owp", bufs=12) as owp,
                tc.tile_pool(name="qkp", bufs=TE) as qkp,
                tc.tile_pool(name="vp", bufs=TS) as vp,
                tc.tile_pool(name="ptp", bufs=9) as ptp,
                tc.tile_pool(name="otp", bufs=14) as otp,
                tc.tile_pool(name="smp", bufs=2) as smp,
                # "ps" (QK/scores; drained fast by ScalarE) and "psav" (AV;
                # drained by the slow DVE softmax-denominator chain) get
                # separate rings so score groups never wait on the chain.
                tc.tile_pool(name="psA", bufs=2, space="PSUM") as psA,
                tc.tile_pool(name="psW", bufs=2, space="PSUM") as psW,
            ):
                # deferred fused-B work from the previous q-half:
                # (h, qh, ot_tiles, ow_tiles)
                pending = []

                def fused_b(h, qh, ots, ows):
                    """out[s0:s0+512, :] += (oT_h/r).T @ ow_h  (+bias at h==0)"""
                    for sti in range(4):
                        st = qh * 4 + sti
                        ps = psW.tile([128, E], F32, tag="psw")
                        if h == 0:
                            for g0, gn in ((0, 512), (512, 256)):
                                nc.tensor.matmul(ps[:, g0:g0 + gn], onesr[:],
                                                 fb[:, g0:g0 + gn],
                                                 start=True, stop=False,
                                                 skip_group_check=True)
                        for et in range(TE):
                            lt = ots[et][:, sti * 128:(sti + 1) * 128]
                            for g0, gn in ((0, 512), (512, 256)):
                                nc.tensor.matmul(
                                    ps[:, g0:g0 + gn], lt, ows[et][:, g0:g0 + gn],
                                    start=(et == 0 and h != 0), stop=(et == TE - 1),
                                    skip_group_check=True,
                                )
                        if h == 0:
                            nc.vector.tensor_copy(acc[:, st, :], ps[:])
                        else:
                            nc.vector.tensor_add(acc[:, st, :], acc[:, st, :], ps[:])
                        if h == H - 1:
                            nc.sync.dma_start(out_d[st * 128:(st + 1) * 128, :],
                                              acc[:, st, :])

                for h in range(H):
                    w = []
                    for et in range(TE):
                        wt = wp.tile([128, F3], BF16, tag="w")
                        src = w_d[h, et * 128:(et + 1) * 128, :]
                        nc.sync.dma_start(wt[:], src)
                        w.append(wt)
                    ows = []
                    for et in range(TE):
                        he = h * TE + et
                        ot_w = owp.tile([128, E], BF16, tag="ow")
                        nc.sync.dma_start(ot_w[:], owT_d[he * 128:(he + 1) * 128, :])
                        ows.append(ot_w)

                    # Q^T (with bias) / K^T (bias dropped: softmax-invariant)
                    qk = {}
                    for part, tag in ((0, "qt"), (1, "kt")):
                        tiles = []
                        for ftl in range(TE):
                            f0 = part * E + ftl * 128
                            dst = qkp.tile([128, S], BF16, tag=tag)
                            for sc in range(2):
                                ps = psA.tile([128, 512], F32, tag="ps")
                                for et in range(TE):
                                    nc.tensor.matmul(
                                        ps[:],
                                        w[et][:, f0:f0 + 128],
                                        xt[:, et, sc * 512:(sc + 1) * 512],
                                        start=(et == 0), stop=(et == TE - 1),
                                    )
                                if part == 0:
                                    bcol = h * TE + ftl
                                    nc.scalar.add(dst[:, sc * 512:(sc + 1) * 512],
                                                  ps[:], add=qb[:, bcol:bcol + 1])
                                else:
                                    nc.scalar.copy(dst[:, sc * 512:(sc + 1) * 512],
                                                   ps[:])
                            tiles.append(dst)
                        qk[tag] = tiles
                    qt, kt = qk["qt"], qk["kt"]

                    # V projection [k, e]; V bias folded into final bias
                    vtiles = []
                    for st in range(TS):
                        vt = vp.tile([128, E], BF16, tag="v")
                        ps = psW.tile([128, E], F32, tag="psw")
                        for et in range(TE):
                            xs = xt[:, et, st * 128:(st + 1) * 128]
                            for n0, nn in ((0, 512), (512, 256)):
                                nc.tensor.matmul(
                                    ps[:, n0:n0 + nn],
                                    xs,
                                    w[et][:, 2 * E + n0:2 * E + n0 + nn],
                                    start=(et == 0), stop=(et == TE - 1),
                                    skip_group_check=True,
                                )
                        nc.vector.tensor_copy(vt[:], ps[:])
                        vtiles.append(vt)

                    for qh in range(2):
                        q0 = qh * 512
                        # scores^T + exp
                        pts = []
                        for kti in range(TS):
                            ps = psA.tile([128, 512], F32, tag="ps")
                            for et in range(TE):
                                nc.tensor.matmul(
                                    ps[:],
                                    kt[et][:, kti * 128:(kti + 1) * 128],
                                    qt[et][:, q0:q0 + 512],
                                    start=(et == 0), stop=(et == TE - 1),
                                )
                            pt = ptp.tile([128, 512], BF16, tag="pt")
                            nc.scalar.activation(pt[:], ps[:], Exp, scale=SCALE)
                            pts.append(pt)
                        # softmax denominators r[q] off the PE:
                        # DVE tree-sum + GPSIMD partition all-reduce + DVE recip
                        tsum = smp.tile([128, 512], F32, tag="tsum")
                        nc.vector.tensor_add(tsum[:], pts[0][:], pts[1][:])
                        for kti in range(2, TS):
                            nc.vector.tensor_add(tsum[:], tsum[:], pts[kti][:])
                        rall = smp.tile([128, 512], F32, tag="rall")
                        nc.gpsimd.partition_all_reduce(rall[:], tsum[:], 128,
                                                       bass_isa.ReduceOp.add)
                        rb = smp.tile([128, 512], F32, tag="rb")
                        nc.vector.reciprocal(rb[:], rall[:])

                        ots = []
                        for et in range(TE):
                            ps = psA.tile([128, 512], F32, tag="psav")
                            for kti in range(TS):
                                nc.tensor.matmul(
                                    ps[:],
                                    vtiles[kti][:, et * 128:(et + 1) * 128],
                                    pts[kti][:],
                                    start=(kti == 0), stop=(kti == TS - 1),
                                )
                            ot = otp.tile([128, 512], BF16, tag="ot")
                            nc.vector.tensor_mul(ot[:], ps[:], rb[:])
                            ots.append(ot)
                        # drain the previous q-half's fused projection after
                        # this half's AV block so the softmax-denominator
                        # chain (DVE tree + GPSIMD + recip + muls) has a full
                        # stage of PE work as cover
                        if pending:
                            fused_b(*pending.pop())
                        pending.append((h, qh, ots, ows))

                for item in pending:
                    fused_b(*item)

    nc.compile()
    return nc


def _get_built():
    global _BUILT
    if _BUILT is None:
        _BUILT = _build()
    return _BUILT


def kernel(x, qkv_w, qkv_b, out_w, out_b):
    import ml_dtypes
    from concourse.bass_utils import run_bass_kernel_spmd

    x = np.asarray(x, np.float32)
    qkv_w = np.asarray(qkv_w, np.float32)
    qkv_b = np.asarray(qkv_b, np.float32)
    out_w = np.asarray(out_w, np.float32)
    out_b = np.asarray(out_b, np.float32)

    bf16 = ml_dtypes.bfloat16
    xT_all = np.ascontiguousarray(x.transpose(0, 2, 1)).astype(bf16)  # [B,E,S]
    wqkvT = np.ascontiguousarray(qkv_w.transpose(0, 2, 1)).astype(bf16)
    owT = np.ascontiguousarray(out_w.T).astype(bf16)                  # [HE, E]
    # Q bias only, laid out [128, h*TE+ftl]
    qb = np.ascontiguousarray(
        qkv_b[:, :E].reshape(H, TE, 128).transpose(2, 0, 1).reshape(128, H * TE)
    )
    bv_cat = qkv_b[:, 2 * E:].reshape(HE)
    fb = (out_b + out_w @ bv_cat).reshape(1, E).astype(bf16)

    shared = {
        "wqkvT": wqkvT,
        "owT": owT,
        "qb": qb,
        "fb": fb,
        "onesr": np.ones((1, 128), bf16),
    }
    in_maps = [dict(shared, xT=xT_all[c]) for c in range(B)]

    nc = _get_built()
    res = run_bass_kernel_spmd(nc, in_maps, list(range(B)), trace=TRACE)
    if TRACE:
        global LAST_EXEC_TIME_NS, LAST_TRACE
        LAST_EXEC_TIME_NS = res.exec_time_ns
        LAST_TRACE = res.instructions_and_trace
    return np.stack([res.results[c]["out"] for c in range(B)], axis=0)


TRACE = False
LAST_EXEC_TIME_NS = None
LAST_TRACE = None
